# revision 18
# baseline (speedup 1.0000x reference)
"""Trainium2 Bass kernel for nn_BinaryLinear (XNOR-net style binary linear).

reference:
    bx = sign(x) * mean(|x|); bw = sign(w) * mean(|w|); bb = sign(b) * mean(|b|)
    y = bx @ bw.T + bb          x:[8192,4096] w:[4096,4096] b:[4096]

Identity used on device:
    y = c * (sign(x) @ sign(w).T) + sb * sign(b),   c = mean|x| * mean|w|

Sharding: data-parallel over rows of x (1024/core).  The weight is NOT
replicated: each core receives only a distinct 512-row slice of w
("wshard"), signs + transposes it to a compact fp8 tensor (2.1MB), and
one 8-core AllGather distributes all eight blocks to every core --
cutting per-core HBM traffic from ~109MB to ~80MB and making the
per-block steady-state a single 2.1MB load.  One AllReduce of
[sum|x|_part, sum|w|_part] produces the global scale concurrently.

Engine plan: PE runs 1024 fp8 DoubleRow matmuls (K=256 each, 2x bf16
throughput; +-1 is exact in e4m3 and products accumulate exactly in
f32 PSUM).  Signs are computed by ACT (f32 -> bf16 +-1), transposed through the
DMA xbar (2-byte), then cast bf16 -> fp8 by DVE into the [k-tile,
free] layout DoubleRow consumes directly.
Queues: gpsimd(SWDGE)=x/wsh loads + out stores + collective, sync=xbar
transposes only, scalar ring=w loads, ACT=signs, DVE=reduces+casts+epilogue.
"""

import sys

for _p in ("/opt/trn_rl_repo", "/opt/pypackages"):
    if _p not in sys.path:
        sys.path.insert(0, _p)

import numpy as np

import concourse.bass as bass
import concourse.bass_isa as bass_isa
import concourse.mybir as mybir
import concourse.tile as tile
from concourse import bacc
from concourse.bass import ds, ts
from concourse.bass_utils import run_bass_kernel_spmd

N, IN, OUT = 8192, 4096, 4096
NCORES = 8
NSH = N // NCORES          # 1024 rows of x per core
WSH = OUT // NCORES        # 512 rows of w per core (for the |w| reduction)
P = 128

F32 = mybir.dt.float32
BF16 = mybir.dt.bfloat16

# mean = sum * 2^-k; all counts are powers of two so the scaling is exact.
X_SCALE = 1.0 / float(N * IN)          # 2^-25
W_SCALE = 1.0 / float(OUT * IN)        # 2^-24
B_SCALE = 1.0 / float(OUT)             # 2^-12


def build_kernel():
    nc = bacc.Bacc("TRN2", target_bir_lowering=False, debug=False, num_devices=NCORES)

    x = nc.dram_tensor("x", [NSH, IN], F32, kind="ExternalInput").ap()
    wsh = nc.dram_tensor("wsh", [WSH, IN], F32, kind="ExternalInput").ap()
    b = nc.dram_tensor("b", [OUT], F32, kind="ExternalInput").ap()
    out = nc.dram_tensor("out", [NSH, OUT], F32, kind="ExternalOutput").ap()

    cc_in = nc.dram_tensor("cc_in", [1, 2], F32)
    cc_out = nc.dram_tensor("cc_out", [1, 2], F32, addr_space="Shared")
    WTSZ = 128 * (IN // 128) * 512        # fp8 elements per w block
    wt_in0 = nc.dram_tensor("wt_in0", [1, WTSZ // 2], mybir.dt.float8e4)
    wt_in1 = nc.dram_tensor("wt_in1", [1, WTSZ // 2], mybir.dt.float8e4)
    wt_all0 = nc.dram_tensor("wt_all0", [NCORES, WTSZ // 2], mybir.dt.float8e4,
                             addr_space="Shared")
    wt_all1 = nc.dram_tensor("wt_all1", [NCORES, WTSZ // 2], mybir.dt.float8e4,
                             addr_space="Shared")

    NKT = IN // P              # 32 k-tiles
    NMT = NSH // P             # 8 m-tiles
    NOB = OUT // 512           # 8 output column blocks

    with tile.TileContext(nc) as tc:
        with (
            tc.tile_pool(name="xt", bufs=1) as xtp,
            tc.tile_pool(name="xslab", bufs=4) as xsp,
            tc.tile_pool(name="wshp", bufs=3) as whp,
            tc.tile_pool(name="sgn", bufs=4) as sgp,
            tc.tile_pool(name="tstage", bufs=4) as tsp,
            tc.tile_pool(name="stats", bufs=1) as stp,
            tc.tile_pool(name="wt", bufs=2) as wtp,
            tc.tile_pool(name="ost", bufs=3) as osp,
            tc.tile_pool(name="mm_psum", bufs=8, space="PSUM") as mmp,
        ):
            FP8 = mybir.dt.float8e4

            # sign(x)^T resident: [i-within-tile, k-tile, n] (fp8)
            XT = xtp.tile([P, NKT, NSH], FP8)

            xstats = stp.tile([P, 16], F32)
            wstats = stp.tile([P, 8], F32)
            spair = stp.tile([P, 2], F32)
            sred = stp.tile([P, 2], F32)
            g = stp.tile([1, 2], F32)
            t0 = stp.tile([1, 1], F32)
            c1 = stp.tile([1, 1], F32)
            c_col = stp.tile([P, 1], F32)
            browb = stp.tile([1, OUT], BF16)
            babs = stp.tile([1, 1], F32)
            sb = stp.tile([1, 1], F32)
            bias_bcast = stp.tile([P, OUT], BF16)

            # ---- all input loads issue back-to-back on sync first, so
            # transfers pipeline through the SDMA engines at full HBM rate
            wslabs = []
            for s2 in range(8):
                sr, ch = s2 // 2, s2 % 2
                wss = whp.tile([P, 2048], F32)
                nc.gpsimd.dma_start(wss[:], wsh[ts(sr, P), ts(ch, 2048)])
                wslabs.append(wss)
            xslabs = []
            for s2 in range(16):
                sr, ch = s2 // 2, s2 % 2
                xslab = xsp.tile([P, 2048], F32)
                nc.gpsimd.dma_start(xslab[:], x[ts(sr, P), ts(ch, 2048)])
                xslabs.append(xslab)

            # ---- this core's w block: |w| partial + sign^T -> fp8, then
            # store to DRAM and AllGather all eight blocks (rank == block)
            WTloc = xtp.tile([P, NKT, 512], FP8, tag="wtloc")
            for s2 in range(8):
                sr, ch = s2 // 2, s2 % 2
                wss = wslabs[s2]
                nc.vector.tensor_reduce(
                    wstats[:, ds(s2, 1)],
                    wss[:],
                    axis=mybir.AxisListType.X,
                    op=mybir.AluOpType.add,
                    apply_absolute_value=True,
                )
                sgn = sgp.tile([P, 2048], BF16, tag="sgn")
                nc.scalar.sign(sgn[:], wss[:])
                tst = tsp.tile([P, 16, P], BF16, tag="tstage")
                nc.sync.dma_start_transpose(tst[:], sgn[:])
                nc.vector.tensor_copy(
                    WTloc[:, ds(ch * 16, 16), ts(sr, P)], tst[:]
                )
            # two halves -> two small AllGathers (mesh regime, lower latency)
            nc.gpsimd.dma_start(
                wt_in0.rearrange("a (p q) -> (a p) q", p=P),
                WTloc[:, 0 : NKT // 2, :],
            )
            nc.gpsimd.dma_start(
                wt_in1.rearrange("a (p q) -> (a p) q", p=P),
                WTloc[:, NKT // 2 : NKT, :],
            )
            nc.gpsimd.collective_compute(
                "AllGather",
                mybir.AluOpType.bypass,
                replica_groups=[list(range(NCORES))],
                ins=[wt_in0[:]],
                outs=[wt_all0[:]],
            )
            nc.gpsimd.collective_compute(
                "AllGather",
                mybir.AluOpType.bypass,
                replica_groups=[list(range(NCORES))],
                ins=[wt_in1[:]],
                outs=[wt_all1[:]],
            )

            # ---- x -> sign(x)^T: ACT sign -> bf16, xbar transpose, cast fp8
            for s2 in range(16):
                sr, ch = s2 // 2, s2 % 2
                xslab = xslabs[s2]
                nc.vector.tensor_reduce(
                    xstats[:, ds(s2, 1)],
                    xslab[:],
                    axis=mybir.AxisListType.X,
                    op=mybir.AluOpType.add,
                    apply_absolute_value=True,
                )
                sgn = sgp.tile([P, 2048], BF16)
                nc.scalar.sign(sgn[:], xslab[:])
                tst = tsp.tile([P, 16, P], BF16)
                nc.sync.dma_start_transpose(tst[:], sgn[:])
                nc.vector.tensor_copy(XT[:, ds(ch * 16, 16), ts(sr, P)], tst[:])

            # ---- global scale c = (sum|x| * sum|w|) * 2^-49 via AllReduce
            nc.vector.tensor_reduce(
                spair[:, 0:1], xstats[:], axis=mybir.AxisListType.X,
                op=mybir.AluOpType.add,
            )
            nc.vector.tensor_reduce(
                spair[:, 1:2], wstats[:], axis=mybir.AxisListType.X,
                op=mybir.AluOpType.add,
            )
            nc.gpsimd.partition_all_reduce(
                sred[:], spair[:], channels=P, reduce_op=bass_isa.ReduceOp.add
            )
            nc.gpsimd.dma_start(cc_in[:], sred[0:1, :])
            nc.gpsimd.collective_compute(
                "AllReduce",
                mybir.AluOpType.add,
                replica_groups=[list(range(NCORES))],
                ins=[cc_in[:]],
                outs=[cc_out[:]],
            )
            nc.gpsimd.dma_start(g[:], cc_out[:])
            nc.vector.tensor_tensor(
                t0[:], g[:, 0:1], g[:, 1:2], mybir.AluOpType.mult
            )
            nc.scalar.mul(c1[:], t0[:], X_SCALE * W_SCALE)
            nc.gpsimd.partition_broadcast(c_col[:], c1[:])

            # ---- bias row: sb*sign(b) broadcast to all partitions (bf16)
            nc.gpsimd.dma_start(browb[:], b.rearrange("(a o) -> a o", a=1))
            nc.vector.tensor_reduce(
                babs[:], browb[:], axis=mybir.AxisListType.X,
                op=mybir.AluOpType.add, apply_absolute_value=True,
            )
            nc.scalar.mul(sb[:], babs[:], B_SCALE)
            nc.scalar.sign(browb[:], browb[:])
            nc.scalar.mul(browb[:], browb[:], sb[:])
            nc.gpsimd.partition_broadcast(bias_bcast[:], browb[:])

            # ---- main: per 512-col block, one 2.1MB fp8 load + matmuls
            for ob in range(NOB):
                WT = wtp.tile([P, NKT, 512], FP8)
                nc.scalar.dma_start(
                    WT[:, 0 : NKT // 2, :],
                    wt_all0[ds(ob, 1), :].rearrange("a (p q) -> (a p) q", p=P),
                )
                nc.scalar.dma_start(
                    WT[:, NKT // 2 : NKT, :],
                    wt_all1[ds(ob, 1), :].rearrange("a (p q) -> (a p) q", p=P),
                )
                for m in range(NMT):
                    ps = mmp.tile([P, 512], F32)
                    for k2 in range(0, NKT, 2):
                        nc.tensor.matmul(
                            ps[:],
                            XT[:, ds(k2, 2), ts(m, P)],
                            WT[:, ds(k2, 2), :],
                            start=(k2 == 0),
                            stop=(k2 == NKT - 2),
                            perf_mode=mybir.MatmulPerfMode.DoubleRow,
                        )
                    ost = osp.tile([P, 512], F32)
                    nc.vector.scalar_tensor_tensor(
                        ost[:],
                        ps[:],
                        c_col[:],
                        bias_bcast[:, ds(ob * 512, 512)],
                        op0=mybir.AluOpType.mult,
                        op1=mybir.AluOpType.add,
                    )
                    nc.gpsimd.dma_start(out[ts(m, P), ds(ob * 512, 512)], ost[:])

    nc.compile()
    return nc


_NC_CACHE = None


def _get_nc():
    global _NC_CACHE
    if _NC_CACHE is None:
        _NC_CACHE = build_kernel()
    return _NC_CACHE


def make_in_maps(x, weight, bias):
    x = np.ascontiguousarray(x, dtype=np.float32)
    weight = np.ascontiguousarray(weight, dtype=np.float32)
    bias = np.ascontiguousarray(bias, dtype=np.float32)
    in_maps = []
    for c in range(NCORES):
        in_maps.append(
            {
                "x": x[c * NSH : (c + 1) * NSH],
                "wsh": np.ascontiguousarray(weight[c * WSH : (c + 1) * WSH]),
                "b": bias,
            }
        )
    return in_maps


def kernel(x, weight, bias):
    nc = _get_nc()
    res = run_bass_kernel_spmd(nc, make_in_maps(x, weight, bias), list(range(NCORES)))
    return np.concatenate([res.results[c]["out"] for c in range(NCORES)], axis=0)


if __name__ == "__main__":
    xs = np.random.randn(N, IN).astype(np.float32)
    ws = np.random.uniform(-1, 1, (OUT, IN)).astype(np.float32) * (1.0 / np.sqrt(IN * OUT))
    bs = np.random.uniform(-1, 1, (OUT,)).astype(np.float32) * (1.0 / np.sqrt(IN * OUT))
    y = kernel(xs, ws, bs)
    sx = np.abs(xs).mean(dtype=np.float64)
    sw = np.abs(ws).mean(dtype=np.float64)
    sbv = np.abs(bs).mean(dtype=np.float64)
    ref = (sx * sw) * (np.sign(xs) @ np.sign(ws).T) + sbv * np.sign(bs)
    err = np.abs(y - ref).max() / np.abs(ref).max()
    print("quick rel err:", err)


# revision 19
# speedup vs baseline: 1.2794x; 1.2794x over previous
"""Trainium2 Bass kernel for nn_BinaryLinear (XNOR-net style binary linear).

reference:
    bx = sign(x) * mean(|x|); bw = sign(w) * mean(|w|); bb = sign(b) * mean(|b|)
    y = bx @ bw.T + bb          x:[8192,4096] w:[4096,4096] b:[4096]

Identity used on device:
    y = c * (sign(x) @ sign(w).T) + sb * sign(b),   c = mean|x| * mean|w|

Sharding: data-parallel over rows of x (1024/core).  The weight is NOT
replicated: each core receives only a distinct 512-row slice of w
("wshard"), signs + transposes it to a compact fp8 tensor (2.1MB), and
one 8-core AllGather distributes all eight blocks to every core --
cutting per-core HBM traffic from ~109MB to ~80MB and making the
per-block steady-state a single 2.1MB load.  One AllReduce of
[sum|x|_part, sum|w|_part] produces the global scale concurrently.

Engine plan: PE runs 1024 fp8 DoubleRow matmuls (K=256 each, 2x bf16
throughput; +-1 is exact in e4m3 and products accumulate exactly in
f32 PSUM).  Signs are computed by ACT (f32 -> bf16 +-1), transposed through the
DMA xbar (2-byte), then cast bf16 -> fp8 by DVE into the [k-tile,
free] layout DoubleRow consumes directly.
Queues: gpsimd(SWDGE)=x/wsh loads + out stores + collective, sync=xbar
transposes only, scalar ring=w loads, ACT=signs, DVE=reduces+casts+epilogue.
"""

import sys

for _p in ("/opt/trn_rl_repo", "/opt/pypackages"):
    if _p not in sys.path:
        sys.path.insert(0, _p)

import numpy as np

import concourse.bass as bass
import concourse.bass_isa as bass_isa
import concourse.mybir as mybir
import concourse.tile as tile
from concourse import bacc
from concourse.bass import ds, ts
from concourse.bass_utils import run_bass_kernel_spmd

N, IN, OUT = 8192, 4096, 4096
NCORES = 8
NSH = N // NCORES          # 1024 rows of x per core
WSH = OUT // NCORES        # 512 rows of w per core (for the |w| reduction)
P = 128

F32 = mybir.dt.float32
BF16 = mybir.dt.bfloat16

# mean = sum * 2^-k; all counts are powers of two so the scaling is exact.
X_SCALE = 1.0 / float(N * IN)          # 2^-25
W_SCALE = 1.0 / float(OUT * IN)        # 2^-24
B_SCALE = 1.0 / float(OUT)             # 2^-12


def build_kernel():
    nc = bacc.Bacc("TRN2", target_bir_lowering=False, debug=False, num_devices=NCORES)

    x = nc.dram_tensor("x", [NSH, IN], F32, kind="ExternalInput").ap()
    wsh = nc.dram_tensor("wsh", [WSH, IN], F32, kind="ExternalInput").ap()
    b = nc.dram_tensor("b", [OUT], F32, kind="ExternalInput").ap()
    out = nc.dram_tensor("out", [NSH, OUT], F32, kind="ExternalOutput").ap()

    cc_in = nc.dram_tensor("cc_in", [1, 2], F32)
    cc_out = nc.dram_tensor("cc_out", [1, 2], F32, addr_space="Shared")
    WTSZ = 128 * (IN // 128) * 512        # fp8 elements per w block
    wt_in0 = nc.dram_tensor("wt_in0", [1, WTSZ // 2], mybir.dt.float8e4)
    wt_in1 = nc.dram_tensor("wt_in1", [1, WTSZ // 2], mybir.dt.float8e4)
    wt_all0 = nc.dram_tensor("wt_all0", [NCORES, WTSZ // 2], mybir.dt.float8e4,
                             addr_space="Shared")
    wt_all1 = nc.dram_tensor("wt_all1", [NCORES, WTSZ // 2], mybir.dt.float8e4,
                             addr_space="Shared")

    NKT = IN // P              # 32 k-tiles
    NMT = NSH // P             # 8 m-tiles
    NOB = OUT // 512           # 8 output column blocks

    with tile.TileContext(nc) as tc:
        with (
            tc.tile_pool(name="xt", bufs=1) as xtp,
            tc.tile_pool(name="xslab", bufs=3) as xsp,
            tc.tile_pool(name="sgn", bufs=3) as sgp,
            tc.tile_pool(name="tstage", bufs=3) as tsp,
            tc.tile_pool(name="stats", bufs=1) as stp,
            tc.tile_pool(name="wt", bufs=2) as wtp,
            tc.tile_pool(name="ost", bufs=3) as osp,
            tc.tile_pool(name="mm_psum", bufs=8, space="PSUM") as mmp,
        ):
            FP8 = mybir.dt.float8e4

            # sign(x)^T resident: [i-within-tile, k-tile, n] (fp8)
            XT = xtp.tile([P, NKT, NSH], FP8)

            xstats = stp.tile([P, 8], F32)
            wstats = stp.tile([P, 4], F32)
            spair = stp.tile([P, 2], F32)
            sred = stp.tile([P, 2], F32)
            g = stp.tile([1, 2], F32)
            t0 = stp.tile([1, 1], F32)
            c1 = stp.tile([1, 1], F32)
            c_col = stp.tile([P, 1], F32)
            browb = stp.tile([1, OUT], BF16)
            babs = stp.tile([1, 1], F32)
            sb = stp.tile([1, 1], F32)
            bias_bcast = stp.tile([P, OUT], BF16)

            # ---- all input loads issue back-to-back on sync first, so
            # transfers pipeline through the SDMA engines at full HBM rate
            wslabs = []
            for sr in range(4):
                wss = xsp.tile([P, IN], F32, tag="xslab")
                nc.sync.dma_start(wss[:], wsh[ts(sr, P), :])
                wslabs.append(wss)
            xslabs = []
            for sr in range(8):
                xslab = xsp.tile([P, IN], F32, tag="xslab")
                nc.sync.dma_start(xslab[:], x[ts(sr, P), :])
                xslabs.append(xslab)

            # ---- this core's w block: |w| partial + sign^T -> fp8, then
            # store to DRAM and AllGather all eight blocks (rank == block)
            WTloc = xtp.tile([P, NKT, 512], FP8, tag="wtloc")
            for sr in range(4):
                wss = wslabs[sr]
                nc.vector.tensor_reduce(
                    wstats[:, ds(sr, 1)],
                    wss[:],
                    axis=mybir.AxisListType.X,
                    op=mybir.AluOpType.add,
                    apply_absolute_value=True,
                )
                sgn = sgp.tile([P, IN], BF16, tag="sgn")
                nc.scalar.sign(sgn[:], wss[:])
                tst = tsp.tile([P, NKT, P], BF16, tag="tstage")
                nc.sync.dma_start_transpose(tst[:], sgn[:])
                nc.vector.tensor_copy(WTloc[:, :, ts(sr, P)], tst[:])
            # two halves -> two small AllGathers (mesh regime, lower latency)
            nc.gpsimd.dma_start(
                wt_in0.rearrange("a (p q) -> (a p) q", p=P),
                WTloc[:, 0 : NKT // 2, :],
            )
            nc.gpsimd.dma_start(
                wt_in1.rearrange("a (p q) -> (a p) q", p=P),
                WTloc[:, NKT // 2 : NKT, :],
            )
            nc.gpsimd.collective_compute(
                "AllGather",
                mybir.AluOpType.bypass,
                replica_groups=[list(range(NCORES))],
                ins=[wt_in0[:]],
                outs=[wt_all0[:]],
            )
            nc.gpsimd.collective_compute(
                "AllGather",
                mybir.AluOpType.bypass,
                replica_groups=[list(range(NCORES))],
                ins=[wt_in1[:]],
                outs=[wt_all1[:]],
            )

            # ---- x -> sign(x)^T: ACT sign -> bf16, xbar transpose, cast fp8
            for sr in range(8):
                xslab = xslabs[sr]
                nc.vector.tensor_reduce(
                    xstats[:, ds(sr, 1)],
                    xslab[:],
                    axis=mybir.AxisListType.X,
                    op=mybir.AluOpType.add,
                    apply_absolute_value=True,
                )
                sgn = sgp.tile([P, IN], BF16, tag="sgn")
                nc.scalar.sign(sgn[:], xslab[:])
                tst = tsp.tile([P, NKT, P], BF16, tag="tstage")
                nc.sync.dma_start_transpose(tst[:], sgn[:])
                nc.vector.tensor_copy(XT[:, :, ts(sr, P)], tst[:])

            # ---- global scale c = (sum|x| * sum|w|) * 2^-49 via AllReduce
            nc.vector.tensor_reduce(
                spair[:, 0:1], xstats[:], axis=mybir.AxisListType.X,
                op=mybir.AluOpType.add,
            )
            nc.vector.tensor_reduce(
                spair[:, 1:2], wstats[:], axis=mybir.AxisListType.X,
                op=mybir.AluOpType.add,
            )
            nc.gpsimd.partition_all_reduce(
                sred[:], spair[:], channels=P, reduce_op=bass_isa.ReduceOp.add
            )
            nc.gpsimd.dma_start(cc_in[:], sred[0:1, :])
            nc.gpsimd.collective_compute(
                "AllReduce",
                mybir.AluOpType.add,
                replica_groups=[list(range(NCORES))],
                ins=[cc_in[:]],
                outs=[cc_out[:]],
            )
            nc.gpsimd.dma_start(g[:], cc_out[:])
            nc.vector.tensor_tensor(
                t0[:], g[:, 0:1], g[:, 1:2], mybir.AluOpType.mult
            )
            nc.scalar.mul(c1[:], t0[:], X_SCALE * W_SCALE)
            nc.gpsimd.partition_broadcast(c_col[:], c1[:])

            # ---- bias row: sb*sign(b) broadcast to all partitions (bf16)
            nc.gpsimd.dma_start(browb[:], b.rearrange("(a o) -> a o", a=1))
            nc.vector.tensor_reduce(
                babs[:], browb[:], axis=mybir.AxisListType.X,
                op=mybir.AluOpType.add, apply_absolute_value=True,
            )
            nc.scalar.mul(sb[:], babs[:], B_SCALE)
            nc.scalar.sign(browb[:], browb[:])
            nc.scalar.mul(browb[:], browb[:], sb[:])
            nc.gpsimd.partition_broadcast(bias_bcast[:], browb[:])

            # ---- main: per 512-col block, one 2.1MB fp8 load + matmuls
            for ob in range(NOB):
                WT = wtp.tile([P, NKT, 512], FP8)
                nc.scalar.dma_start(
                    WT[:, 0 : NKT // 2, :],
                    wt_all0[ds(ob, 1), :].rearrange("a (p q) -> (a p) q", p=P),
                )
                nc.scalar.dma_start(
                    WT[:, NKT // 2 : NKT, :],
                    wt_all1[ds(ob, 1), :].rearrange("a (p q) -> (a p) q", p=P),
                )
                for m in range(NMT):
                    ps = mmp.tile([P, 512], F32)
                    for k2 in range(0, NKT, 2):
                        nc.tensor.matmul(
                            ps[:],
                            XT[:, ds(k2, 2), ts(m, P)],
                            WT[:, ds(k2, 2), :],
                            start=(k2 == 0),
                            stop=(k2 == NKT - 2),
                            perf_mode=mybir.MatmulPerfMode.DoubleRow,
                        )
                    ost = osp.tile([P, 512], F32)
                    nc.vector.scalar_tensor_tensor(
                        ost[:],
                        ps[:],
                        c_col[:],
                        bias_bcast[:, ds(ob * 512, 512)],
                        op0=mybir.AluOpType.mult,
                        op1=mybir.AluOpType.add,
                    )
                    nc.gpsimd.dma_start(out[ts(m, P), ds(ob * 512, 512)], ost[:])

    nc.compile()
    return nc


_NC_CACHE = None


def _get_nc():
    global _NC_CACHE
    if _NC_CACHE is None:
        _NC_CACHE = build_kernel()
    return _NC_CACHE


def make_in_maps(x, weight, bias):
    x = np.ascontiguousarray(x, dtype=np.float32)
    weight = np.ascontiguousarray(weight, dtype=np.float32)
    bias = np.ascontiguousarray(bias, dtype=np.float32)
    in_maps = []
    for c in range(NCORES):
        in_maps.append(
            {
                "x": x[c * NSH : (c + 1) * NSH],
                "wsh": np.ascontiguousarray(weight[c * WSH : (c + 1) * WSH]),
                "b": bias,
            }
        )
    return in_maps


def kernel(x, weight, bias):
    nc = _get_nc()
    res = run_bass_kernel_spmd(nc, make_in_maps(x, weight, bias), list(range(NCORES)))
    return np.concatenate([res.results[c]["out"] for c in range(NCORES)], axis=0)


if __name__ == "__main__":
    xs = np.random.randn(N, IN).astype(np.float32)
    ws = np.random.uniform(-1, 1, (OUT, IN)).astype(np.float32) * (1.0 / np.sqrt(IN * OUT))
    bs = np.random.uniform(-1, 1, (OUT,)).astype(np.float32) * (1.0 / np.sqrt(IN * OUT))
    y = kernel(xs, ws, bs)
    sx = np.abs(xs).mean(dtype=np.float64)
    sw = np.abs(ws).mean(dtype=np.float64)
    sbv = np.abs(bs).mean(dtype=np.float64)
    ref = (sx * sw) * (np.sign(xs) @ np.sign(ws).T) + sbv * np.sign(bs)
    err = np.abs(y - ref).max() / np.abs(ref).max()
    print("quick rel err:", err)


# revision 21
# speedup vs baseline: 1.4729x; 1.1512x over previous
"""Trainium2 Bass kernel for nn_BinaryLinear (XNOR-net style binary linear).

reference:
    bx = sign(x) * mean(|x|); bw = sign(w) * mean(|w|); bb = sign(b) * mean(|b|)
    y = bx @ bw.T + bb          x:[8192,4096] w:[4096,4096] b:[4096]

Identity used on device:
    y = c * (sign(x) @ sign(w).T) + sb * sign(b),   c = mean|x| * mean|w|

Sharding: data-parallel over rows of x (1024/core).  The weight is NOT
replicated: each core receives only a distinct 512-row slice of w
("wshard"), signs + transposes it to a compact fp8 tensor (2.1MB), and
one 8-core AllGather distributes all eight blocks to every core --
cutting per-core HBM traffic from ~109MB to ~80MB and making the
per-block steady-state a single 2.1MB load.  One AllReduce of
[sum|x|_part, sum|w|_part] produces the global scale concurrently.

Engine plan: PE runs 1024 fp8 DoubleRow matmuls (K=256 each, 2x bf16
throughput; +-1 is exact in e4m3 and products accumulate exactly in
f32 PSUM).  Input slabs are transposed on the idle TensorEngine (f32 transpose
mode) before the matmul stream begins; ACT evicts each PSUM transpose
with a fused sign() straight to fp8 in the [k-tile, free] layout that
DoubleRow consumes.  Queues: sync=input loads, scalar ring=per-block
WT loads, gpsimd=stores+collectives, DVE=reduces+epilogue.
"""

import sys

for _p in ("/opt/trn_rl_repo", "/opt/pypackages"):
    if _p not in sys.path:
        sys.path.insert(0, _p)

import numpy as np

import concourse.bass as bass
import concourse.bass_isa as bass_isa
import concourse.mybir as mybir
import concourse.tile as tile
from concourse import bacc
from concourse.bass import ds, ts
from concourse.bass_utils import run_bass_kernel_spmd
from concourse.masks import make_identity

N, IN, OUT = 8192, 4096, 4096
NCORES = 8
NSH = N // NCORES          # 1024 rows of x per core
WSH = OUT // NCORES        # 512 rows of w per core (for the |w| reduction)
P = 128

F32 = mybir.dt.float32
BF16 = mybir.dt.bfloat16

# mean = sum * 2^-k; all counts are powers of two so the scaling is exact.
X_SCALE = 1.0 / float(N * IN)          # 2^-25
W_SCALE = 1.0 / float(OUT * IN)        # 2^-24
B_SCALE = 1.0 / float(OUT)             # 2^-12


def build_kernel():
    nc = bacc.Bacc("TRN2", target_bir_lowering=False, debug=False, num_devices=NCORES)

    x = nc.dram_tensor("x", [NSH, IN], F32, kind="ExternalInput").ap()
    wsh = nc.dram_tensor("wsh", [WSH, IN], F32, kind="ExternalInput").ap()
    b = nc.dram_tensor("b", [OUT], F32, kind="ExternalInput").ap()
    out = nc.dram_tensor("out", [NSH, OUT], F32, kind="ExternalOutput").ap()

    cc_in = nc.dram_tensor("cc_in", [1, 2], F32)
    cc_out = nc.dram_tensor("cc_out", [1, 2], F32, addr_space="Shared")
    WTSZ = 128 * (IN // 128) * 512        # fp8 elements per w block
    wt_in0 = nc.dram_tensor("wt_in0", [1, WTSZ // 2], mybir.dt.float8e4)
    wt_in1 = nc.dram_tensor("wt_in1", [1, WTSZ // 2], mybir.dt.float8e4)
    wt_all0 = nc.dram_tensor("wt_all0", [NCORES, WTSZ // 2], mybir.dt.float8e4,
                             addr_space="Shared")
    wt_all1 = nc.dram_tensor("wt_all1", [NCORES, WTSZ // 2], mybir.dt.float8e4,
                             addr_space="Shared")

    NKT = IN // P              # 32 k-tiles
    NMT = NSH // P             # 8 m-tiles
    NOB = OUT // 512           # 8 output column blocks

    with tile.TileContext(nc) as tc:
        with (
            tc.tile_pool(name="xt", bufs=1) as xtp,
            tc.tile_pool(name="xslab", bufs=5) as xsp,
            tc.tile_pool(name="const", bufs=1) as cst,
            tc.tile_pool(name="stats", bufs=1) as stp,
            tc.tile_pool(name="wt", bufs=2) as wtp,
            tc.tile_pool(name="ost", bufs=3) as osp,
            tc.tile_pool(name="mm_psum", bufs=8, space="PSUM") as mmp,
        ):
            FP8 = mybir.dt.float8e4
            ident = cst.tile([P, P], F32)
            make_identity(nc, ident)

            # sign(x)^T resident: [i-within-tile, k-tile, n] (fp8)
            XT = xtp.tile([P, NKT, NSH], FP8)

            xstats = stp.tile([P, 8], F32)
            wstats = stp.tile([P, 4], F32)
            spair = stp.tile([P, 2], F32)
            sred = stp.tile([P, 2], F32)
            g = stp.tile([1, 2], F32)
            t0 = stp.tile([1, 1], F32)
            c1 = stp.tile([1, 1], F32)
            c_col = stp.tile([P, 1], F32)
            browb = stp.tile([1, OUT], BF16)
            babs = stp.tile([1, 1], F32)
            sb = stp.tile([1, 1], F32)
            bias_bcast = stp.tile([P, OUT], BF16)

            # ---- all input loads issue back-to-back on sync first, so
            # transfers pipeline through the SDMA engines at full HBM rate
            wslabs = []
            for sr in range(4):
                wss = xsp.tile([P, IN], F32, tag="xslab")
                nc.sync.dma_start(wss[:], wsh[ts(sr, P), :])
                wslabs.append(wss)
            xslabs = []
            for sr in range(8):
                xslab = xsp.tile([P, IN], F32, tag="xslab")
                nc.sync.dma_start(xslab[:], x[ts(sr, P), :])
                xslabs.append(xslab)

            # ---- this core's w block: |w| partial + sign^T -> fp8, then
            # store to DRAM and AllGather all eight blocks (rank == block)
            WTloc = xtp.tile([P, NKT, 512], FP8, tag="wtloc")
            for sr in range(4):
                wss = wslabs[sr]
                nc.vector.tensor_reduce(
                    wstats[:, ds(sr, 1)],
                    wss[:],
                    axis=mybir.AxisListType.X,
                    op=mybir.AluOpType.add,
                    apply_absolute_value=True,
                )
                for q in range(NKT // 4):
                    pt = mmp.tile([P, 512], F32, tag="ps")
                    for j in range(4):
                        nc.tensor.transpose(
                            pt[:, ts(j, P)], wss[:, ds(q * 512 + j * P, P)],
                            ident[:],
                        )
                    nc.scalar.sign(
                        WTloc[:, ds(q * 4, 4), ts(sr, P)],
                        pt.rearrange("p (a c) -> p a c", a=4),
                    )
            # two halves -> two small AllGathers (mesh regime, lower latency)
            nc.gpsimd.dma_start(
                wt_in0.rearrange("a (p q) -> (a p) q", p=P),
                WTloc[:, 0 : NKT // 2, :],
            )
            nc.gpsimd.dma_start(
                wt_in1.rearrange("a (p q) -> (a p) q", p=P),
                WTloc[:, NKT // 2 : NKT, :],
            )
            nc.gpsimd.collective_compute(
                "AllGather",
                mybir.AluOpType.bypass,
                replica_groups=[list(range(NCORES))],
                ins=[wt_in0[:]],
                outs=[wt_all0[:]],
            )
            nc.gpsimd.collective_compute(
                "AllGather",
                mybir.AluOpType.bypass,
                replica_groups=[list(range(NCORES))],
                ins=[wt_in1[:]],
                outs=[wt_all1[:]],
            )

            # ---- x -> sign(x)^T: ACT sign -> bf16, xbar transpose, cast fp8
            for sr in range(8):
                xslab = xslabs[sr]
                nc.vector.tensor_reduce(
                    xstats[:, ds(sr, 1)],
                    xslab[:],
                    axis=mybir.AxisListType.X,
                    op=mybir.AluOpType.add,
                    apply_absolute_value=True,
                )
                for q in range(NKT // 4):
                    pt = mmp.tile([P, 512], F32, tag="ps")
                    for j in range(4):
                        nc.tensor.transpose(
                            pt[:, ts(j, P)], xslab[:, ds(q * 512 + j * P, P)],
                            ident[:],
                        )
                    nc.scalar.sign(
                        XT[:, ds(q * 4, 4), ts(sr, P)],
                        pt.rearrange("p (a c) -> p a c", a=4),
                    )

            # ---- global scale c = (sum|x| * sum|w|) * 2^-49 via AllReduce
            nc.vector.tensor_reduce(
                spair[:, 0:1], xstats[:], axis=mybir.AxisListType.X,
                op=mybir.AluOpType.add,
            )
            nc.vector.tensor_reduce(
                spair[:, 1:2], wstats[:], axis=mybir.AxisListType.X,
                op=mybir.AluOpType.add,
            )
            nc.gpsimd.partition_all_reduce(
                sred[:], spair[:], channels=P, reduce_op=bass_isa.ReduceOp.add
            )
            nc.gpsimd.dma_start(cc_in[:], sred[0:1, :])
            nc.gpsimd.collective_compute(
                "AllReduce",
                mybir.AluOpType.add,
                replica_groups=[list(range(NCORES))],
                ins=[cc_in[:]],
                outs=[cc_out[:]],
            )
            nc.gpsimd.dma_start(g[:], cc_out[:])
            nc.vector.tensor_tensor(
                t0[:], g[:, 0:1], g[:, 1:2], mybir.AluOpType.mult
            )
            nc.scalar.mul(c1[:], t0[:], X_SCALE * W_SCALE)
            nc.gpsimd.partition_broadcast(c_col[:], c1[:])

            # ---- bias row: sb*sign(b) broadcast to all partitions (bf16)
            nc.gpsimd.dma_start(browb[:], b.rearrange("(a o) -> a o", a=1))
            nc.vector.tensor_reduce(
                babs[:], browb[:], axis=mybir.AxisListType.X,
                op=mybir.AluOpType.add, apply_absolute_value=True,
            )
            nc.scalar.mul(sb[:], babs[:], B_SCALE)
            nc.scalar.sign(browb[:], browb[:])
            nc.scalar.mul(browb[:], browb[:], sb[:])
            nc.gpsimd.partition_broadcast(bias_bcast[:], browb[:])

            # ---- main: per 512-col block, one 2.1MB fp8 load + matmuls
            for ob in range(NOB):
                WT = wtp.tile([P, NKT, 512], FP8)
                nc.scalar.dma_start(
                    WT[:, 0 : NKT // 2, :],
                    wt_all0[ds(ob, 1), :].rearrange("a (p q) -> (a p) q", p=P),
                )
                nc.scalar.dma_start(
                    WT[:, NKT // 2 : NKT, :],
                    wt_all1[ds(ob, 1), :].rearrange("a (p q) -> (a p) q", p=P),
                )
                for m in range(NMT):
                    ps = mmp.tile([P, 512], F32, tag="ps")
                    for k2 in range(0, NKT, 2):
                        nc.tensor.matmul(
                            ps[:],
                            XT[:, ds(k2, 2), ts(m, P)],
                            WT[:, ds(k2, 2), :],
                            start=(k2 == 0),
                            stop=(k2 == NKT - 2),
                            perf_mode=mybir.MatmulPerfMode.DoubleRow,
                        )
                    ost = osp.tile([P, 512], F32)
                    nc.vector.scalar_tensor_tensor(
                        ost[:],
                        ps[:],
                        c_col[:],
                        bias_bcast[:, ds(ob * 512, 512)],
                        op0=mybir.AluOpType.mult,
                        op1=mybir.AluOpType.add,
                    )
                    nc.gpsimd.dma_start(out[ts(m, P), ds(ob * 512, 512)], ost[:])

    nc.compile()
    return nc


_NC_CACHE = None


def _get_nc():
    global _NC_CACHE
    if _NC_CACHE is None:
        _NC_CACHE = build_kernel()
    return _NC_CACHE


def make_in_maps(x, weight, bias):
    x = np.ascontiguousarray(x, dtype=np.float32)
    weight = np.ascontiguousarray(weight, dtype=np.float32)
    bias = np.ascontiguousarray(bias, dtype=np.float32)
    in_maps = []
    for c in range(NCORES):
        in_maps.append(
            {
                "x": x[c * NSH : (c + 1) * NSH],
                "wsh": np.ascontiguousarray(weight[c * WSH : (c + 1) * WSH]),
                "b": bias,
            }
        )
    return in_maps


def kernel(x, weight, bias):
    nc = _get_nc()
    res = run_bass_kernel_spmd(nc, make_in_maps(x, weight, bias), list(range(NCORES)))
    return np.concatenate([res.results[c]["out"] for c in range(NCORES)], axis=0)


if __name__ == "__main__":
    xs = np.random.randn(N, IN).astype(np.float32)
    ws = np.random.uniform(-1, 1, (OUT, IN)).astype(np.float32) * (1.0 / np.sqrt(IN * OUT))
    bs = np.random.uniform(-1, 1, (OUT,)).astype(np.float32) * (1.0 / np.sqrt(IN * OUT))
    y = kernel(xs, ws, bs)
    sx = np.abs(xs).mean(dtype=np.float64)
    sw = np.abs(ws).mean(dtype=np.float64)
    sbv = np.abs(bs).mean(dtype=np.float64)
    ref = (sx * sw) * (np.sign(xs) @ np.sign(ws).T) + sbv * np.sign(bs)
    err = np.abs(y - ref).max() / np.abs(ref).max()
    print("quick rel err:", err)
